# revision 2
# baseline (speedup 1.0000x reference)
"""Distributed Trainium2 kernel for the diagonal-Rydberg Hamiltonian apply.

Math (n = 22 qubits, dim = 2^22, psi complex as separate real/imag f32):
    out = (rabi/2) * sum_k flip_k(psi) + diag * psi
    diag(b) = sum_k (-detune) * bit_k(b) + sum_{i<j} triu(U,1)[i,j] bit_i(b) bit_j(b)

Distribution: state sharded over 8 cores along the 3 leading qubit axes.
Core d owns amplitudes with global index g = d (top 3 bits). Its output
needs its own shard plus the 3 Hamming-distance-1 partner shards.
All data each core needs is staged in its own DRAM; no collectives.

Per-core layout: local 19 bits -> [128 partitions (bits 12..18), 4096 free
(bits 0..11)]; free axis = 8 chunks of 512 columns (chunk bits 9..11).

Flip-sum strategy (fp8 terms, fp32 PSUM accumulation; the flip term is
small vs the diag term and the rounding noise of 22 independent fp8
terms stays ~1e-3 relative):
  - All whole-tile adds go through FIVE fp8 DoubleRow matmuls per chunk,
    each summing TWO k-tiles in one 512-cycle pass:
      DR1 [A7|I](own_c, own_c^1)   7 partition flips + chunk-bit-0 flip
      DR2 [I|I](own_c^2, own_c^4)  chunk-bit-1/2 flips
      DR3 [I|I](pb0_c, pb1_c)      partner shards d^1, d^2
      DR4 [I|I](pb2_c, P_c)        partner d^4 + partial P = j0+j1+j2+j3
      DR5 [I|I](q_c, p67_c)        partial q = j4+j5+j8, partial p67 = j6+j7
  - Within-chunk free-bit flips j0..j8 are built as bf16 pair-adds on
    DVE (j0..j5, j8) and GPSIMD (j6, j7), stored as fp8 partials into
    spare segments of the same SBUF tensor so they can ride DoubleRow.
  - Diagonal stays exact: D built on-device by a K=9 float32 matmul from
    host bit tables; dx = D (.) x computed on GPSIMD in f32; finalize on
    DVE: out = psum * (rabi/2) + dx.
"""

import os
import sys

import numpy as np
import ml_dtypes

_REPO = "/opt/trn_rl_repo"
if _REPO not in sys.path:
    sys.path.insert(0, _REPO)

import concourse.mybir as mybir  # noqa: E402
from concourse import bacc  # noqa: E402
from concourse import bass  # noqa: E402
from concourse.tile import TileContext  # noqa: E402
from concourse.bass_utils import run_bass_kernel_spmd  # noqa: E402

N_Q = 22
N_GLOBAL = 3
N_CORES = 8
N_LOCAL = N_Q - N_GLOBAL          # 19
P_BITS = 7                        # partition bits (local bits 12..18)
F_BITS = N_LOCAL - P_BITS         # 12 free bits
P = 1 << P_BITS                   # 128
F = 1 << F_BITS                   # 4096
CHUNK = 512
N_CHUNKS = F // CHUNK             # 8
SHARD = P * F                     # 2^19

BF16 = ml_dtypes.bfloat16
FP8 = ml_dtypes.float8_e4m3

# fp8 SBUF tensor segments (units of F=4096 columns)
SEG_OWN, SEG_PB0, SEG_PB1, SEG_PB2, SEG_P, SEG_Q, SEG_P67 = range(7)
N_SEG = 7

_cached = {}


def _pair_ap(t, o1, o2, width=CHUNK):
    """Moving AP [128, 2, width] for a DoubleRow pair: k-tile0 at column o1,
    k-tile1 at column o2 of SBUF tile t. o2 > o1 required."""
    base = t[:, o1:o1 + width]
    d = o2 - o1
    assert d > 0
    return bass.AP(tensor=base.tensor, offset=base.offset,
                   ap=[list(base.ap[0]), [d, 2], [1, width]])


def _build_program():
    """Build the (input-independent) Bass program once per process."""
    if "nc" in _cached:
        return _cached["nc"]

    use_f32r = bool(int(os.environ.get("RYD_F32R", "1")))
    nc = bacc.Bacc("TRN2", num_devices=N_CORES)
    f32, bf16, fp8 = mybir.dt.float32, mybir.dt.bfloat16, mybir.dt.float8e4
    f32r = mybir.dt.float32r
    d_dt = f32r if use_f32r else f32
    Alu = mybir.AluOpType

    x32r = nc.dram_tensor("x32r", [P, F], f32, kind="ExternalInput")
    x32i = nc.dram_tensor("x32i", [P, F], f32, kind="ExternalInput")
    x8r = nc.dram_tensor("x8r", [P, 4 * F], fp8, kind="ExternalInput")
    x8i = nc.dram_tensor("x8i", [P, 4 * F], fp8, kind="ExternalInput")
    xbr = nc.dram_tensor("xbr", [P, F], bf16, kind="ExternalInput")
    xbi = nc.dram_tensor("xbi", [P, F], bf16, kind="ExternalInput")
    wa7i = nc.dram_tensor("wa7i", [P, 2 * P], fp8, kind="ExternalInput")
    wia7 = nc.dram_tensor("wia7", [P, 2 * P], fp8, kind="ExternalInput")
    wii = nc.dram_tensor("wii", [P, 2 * P], fp8, kind="ExternalInput")
    dlhs = nc.dram_tensor("dlhs", [9, P], d_dt, kind="ExternalInput")
    drhs = nc.dram_tensor("drhs", [9, F], d_dt, kind="ExternalInput")
    rh = nc.dram_tensor("rh", [P, 1], f32, kind="ExternalInput")
    outr = nc.dram_tensor("outr", [P, F], f32, kind="ExternalOutput")
    outi = nc.dram_tensor("outi", [P, F], f32, kind="ExternalOutput")

    with TileContext(nc) as tc:
        with (
            tc.tile_pool(name="singles", bufs=1) as singles,
            tc.tile_pool(name="psum", bufs=6, space="PSUM") as psum_pool,
            tc.tile_pool(name="pp", bufs=8) as pp_pool,
            tc.tile_pool(name="dx", bufs=6) as dx_pool,
            tc.tile_pool(name="osb", bufs=6) as osb_pool,
        ):
            # ---- aux loads ----
            t_wa7i = singles.tile([P, 2 * P], fp8, tag="wa7i")
            nc.sync.dma_start(out=t_wa7i[:], in_=wa7i[:])
            t_wia7 = singles.tile([P, 2 * P], fp8, tag="wia7")
            nc.sync.dma_start(out=t_wia7[:], in_=wia7[:])
            t_wii = singles.tile([P, 2 * P], fp8, tag="wii")
            nc.sync.dma_start(out=t_wii[:], in_=wii[:])
            t_dlhs = singles.tile([9, P], d_dt, tag="dlhs")
            nc.sync.dma_start(out=t_dlhs[:], in_=dlhs[:])
            t_drhs = singles.tile([9, F], d_dt, tag="drhs")
            nc.sync.dma_start(out=t_drhs[:], in_=drhs[:])
            t_rh = singles.tile([P, 1], f32, tag="rh")
            nc.sync.dma_start(out=t_rh[:], in_=rh[:])

            # ---- bulk loads, r-component first so its compute starts early
            t_x32, t_x8, t_xb = {}, {}, {}
            for name, d32, d8, db16 in (("r", x32r, x8r, xbr),
                                        ("i", x32i, x8i, xbi)):
                t8 = singles.tile([P, N_SEG * F], fp8, tag=f"x8{name}")
                for s in range(4):
                    nc.sync.dma_start(out=t8[:, s * F:(s + 1) * F],
                                      in_=d8[:, s * F:(s + 1) * F])
                t_x8[name] = t8
                tb = singles.tile([P, F], bf16, tag=f"xb{name}")
                for h in range(2):
                    hs = slice(h * (F // 2), (h + 1) * (F // 2))
                    nc.sync.dma_start(out=tb[:, hs], in_=db16[:, hs])
                t_xb[name] = tb
                t32 = singles.tile([P, F], f32, tag=f"x32{name}")
                for q in range(4):
                    qs = slice(q * (F // 4), (q + 1) * (F // 4))
                    nc.sync.dma_start(out=t32[:, qs], in_=d32[:, qs])
                t_x32[name] = t32

            # ---- diagonal D = dlhs.T @ drhs (K=9), shared by r and i ----
            t_D = singles.tile([P, F], f32, tag="D")
            for c in range(N_CHUNKS):
                sl = slice(c * CHUNK, (c + 1) * CHUNK)
                pd = psum_pool.tile([P, CHUNK], f32, tag="psum")
                nc.tensor.matmul(pd[:], t_dlhs[:], t_drhs[:, sl],
                                 start=True, stop=True)
                nc.scalar.copy(t_D[:, sl], pd[:])

            # DoubleRow stationary views [K, 2, M]
            v_a7i = t_wa7i[:].rearrange("k (two m) -> k two m", two=2)
            v_ia7 = t_wia7[:].rearrange("k (two m) -> k two m", two=2)
            v_ii = t_wii[:].rearrange("k (two m) -> k two m", two=2)
            DR = mybir.MatmulPerfMode.DoubleRow

            # ---- main chunk loop ----
            for name, out_dram in (("r", outr), ("i", outi)):
                x8 = t_x8[name]
                xb = t_xb[name]
                x32 = t_x32[name]
                for c in range(N_CHUNKS):
                    sl = slice(c * CHUNK, (c + 1) * CHUNK)
                    co = c * CHUNK

                    def seg(s, cc=None):
                        return (s * F) + (c if cc is None else cc) * CHUNK

                    def flipv(j):
                        b = 1 << j
                        v = xb[:, sl].rearrange("p (g t b) -> p g t b",
                                                t=2, b=b)
                        return v[:, :, ::-1, :]

                    # within-chunk flip partials (bf16 math, fp8 store)
                    p01 = pp_pool.tile([P, CHUNK], bf16, tag="p01")
                    nc.vector.tensor_add(out=p01[:], in0=flipv(0),
                                         in1=flipv(1))
                    p23 = pp_pool.tile([P, CHUNK], bf16, tag="p23")
                    nc.vector.tensor_add(out=p23[:], in0=flipv(2),
                                         in1=flipv(3))
                    p45 = pp_pool.tile([P, CHUNK], bf16, tag="p45")
                    nc.vector.tensor_add(out=p45[:], in0=flipv(4),
                                         in1=flipv(5))
                    # P = p01 + p23 -> fp8 seg SEG_P
                    nc.vector.tensor_add(
                        out=x8[:, seg(SEG_P):seg(SEG_P) + CHUNK],
                        in0=p01[:], in1=p23[:])
                    # q = p45 + flip8 -> fp8 seg SEG_Q
                    nc.vector.tensor_add(
                        out=x8[:, seg(SEG_Q):seg(SEG_Q) + CHUNK],
                        in0=p45[:], in1=flipv(8))
                    # p67 = flip6 + flip7 -> fp8 seg SEG_P67 (GPSIMD)
                    nc.gpsimd.tensor_add(
                        out=x8[:, seg(SEG_P67):seg(SEG_P67) + CHUNK],
                        in0=flipv(6), in1=flipv(7))

                    # diag product in f32 (GPSIMD)
                    dx = dx_pool.tile([P, CHUNK], f32, tag="dx")
                    nc.gpsimd.tensor_mul(out=dx[:], in0=t_D[:, sl],
                                         in1=x32[:, sl])

                    # ---- five fp8 DoubleRow matmuls ----
                    acc = psum_pool.tile([P, CHUNK], f32, tag="psum")
                    # DR1: A7 on own_c + identity on own_{c^1}
                    c1 = c ^ 1
                    if c < c1:
                        nc.tensor.matmul(acc[:], v_a7i,
                                         _pair_ap(x8, co, c1 * CHUNK),
                                         start=True, stop=False,
                                         perf_mode=DR)
                    else:
                        nc.tensor.matmul(acc[:], v_ia7,
                                         _pair_ap(x8, c1 * CHUNK, co),
                                         start=True, stop=False,
                                         perf_mode=DR)
                    # DR2: own_{c^2} + own_{c^4}
                    ca, cb = sorted((c ^ 2, c ^ 4))
                    nc.tensor.matmul(acc[:], v_ii,
                                     _pair_ap(x8, ca * CHUNK, cb * CHUNK),
                                     start=False, stop=False, perf_mode=DR)
                    # DR3: pb0 + pb1
                    nc.tensor.matmul(acc[:], v_ii,
                                     _pair_ap(x8, seg(SEG_PB0), seg(SEG_PB1)),
                                     start=False, stop=False, perf_mode=DR)
                    # DR4: pb2 + P
                    nc.tensor.matmul(acc[:], v_ii,
                                     _pair_ap(x8, seg(SEG_PB2), seg(SEG_P)),
                                     start=False, stop=False, perf_mode=DR)
                    # DR5: q + p67
                    nc.tensor.matmul(acc[:], v_ii,
                                     _pair_ap(x8, seg(SEG_Q), seg(SEG_P67)),
                                     start=False, stop=True, perf_mode=DR)

                    # finalize: out = acc * (rabi/2) + dx
                    osb = osb_pool.tile([P, CHUNK], f32, tag="osb")
                    nc.vector.scalar_tensor_tensor(
                        out=osb[:], in0=acc[:], scalar=t_rh[:], in1=dx[:],
                        op0=Alu.mult, op1=Alu.add)
                    nc.sync.dma_start(out=out_dram[:, sl], in_=osb[:])

    nc.finalize()
    _cached["nc"] = nc
    return nc


def _host_tables(U, detune, d):
    """Per-core diagonal tables for the K=9 on-device D matmul."""
    Ut = np.triu(U.astype(np.float64), 1)
    gval = {0: (d >> 2) & 1, 1: (d >> 1) & 1, 2: d & 1}  # qubit -> bit of d
    # linear coefficient for every local qubit (3..21)
    lin = np.zeros(N_Q)
    for q in range(3, N_Q):
        lin[q] = -detune + sum(gval[i] * Ut[i, q] for i in range(3))
    const_d = -detune * sum(gval.values())
    for i in range(3):
        for j in range(i + 1, 3):
            const_d += Ut[i, j] * gval[i] * gval[j]

    hi_q = [9 - m for m in range(P_BITS)]        # partition bit m -> qubit
    lo_q = [21 - r for r in range(F_BITS)]       # free bit r -> qubit

    pidx = np.arange(P)
    B7 = ((pidx[:, None] >> np.arange(P_BITS)[None, :]) & 1).astype(np.float64)
    fidx = np.arange(F)
    B12 = ((fidx[:, None] >> np.arange(F_BITS)[None, :]) & 1).astype(np.float64)

    def pair_coeff(qa, qb):
        return Ut[min(qa, qb), max(qa, qb)]

    M_hh = np.zeros((P_BITS, P_BITS))
    for m in range(P_BITS):
        for m2 in range(m + 1, P_BITS):
            M_hh[m, m2] = pair_coeff(hi_q[m], hi_q[m2])
    M_ll = np.zeros((F_BITS, F_BITS))
    for r in range(F_BITS):
        for r2 in range(r + 1, F_BITS):
            M_ll[r, r2] = pair_coeff(lo_q[r], lo_q[r2])
    cross = np.zeros((P_BITS, F_BITS))
    for m in range(P_BITS):
        for r in range(F_BITS):
            cross[m, r] = pair_coeff(hi_q[m], lo_q[r])

    T1 = const_d + B7 @ np.array([lin[q] for q in hi_q]) \
        + np.einsum("pm,mn,pn->p", B7, M_hh, B7)
    T2 = B12 @ np.array([lin[q] for q in lo_q]) \
        + np.einsum("fm,mn,fn->f", B12, M_ll, B12)

    dlhs = np.vstack([B7.T, np.ones((1, P)), T1[None, :]]).astype(np.float32)
    drhs = np.vstack([cross @ B12.T, T2[None, :],
                      np.ones((1, F))]).astype(np.float32)
    return dlhs, drhs


def kernel(state_real, state_imag, rabi, detune, U, n_qubits, **_unused):
    n = int(n_qubits)
    assert n == N_Q, f"kernel hardcoded for {N_Q} qubits, got {n}"
    sr = np.ascontiguousarray(np.asarray(state_real, np.float32)).reshape(
        N_CORES, SHARD)
    si = np.ascontiguousarray(np.asarray(state_imag, np.float32)).reshape(
        N_CORES, SHARD)
    rabi_f = float(np.asarray(rabi).reshape(-1)[0])
    det_f = float(np.asarray(detune).reshape(-1)[0])
    U_np = np.asarray(U, np.float32)

    sr8 = sr.astype(FP8)
    si8 = si.astype(FP8)
    srb = sr.astype(BF16)
    sib = si.astype(BF16)

    pidx = np.arange(P)
    A7 = (np.bitwise_count(pidx[:, None] ^ pidx[None, :]) == 1).astype(FP8)
    I128 = np.eye(P, dtype=FP8)
    wa7i = np.concatenate([A7, I128], axis=1)
    wia7 = np.concatenate([I128, A7], axis=1)
    wii = np.concatenate([I128, I128], axis=1)
    rh_col = np.full((P, 1), rabi_f * 0.5, np.float32)

    in_maps = []
    for d in range(N_CORES):
        dlhs, drhs = _host_tables(U_np, det_f, d)
        in_maps.append({
            "x32r": sr[d].reshape(P, F),
            "x32i": si[d].reshape(P, F),
            "x8r": np.concatenate(
                [sr8[d], sr8[d ^ 1], sr8[d ^ 2], sr8[d ^ 4]]
            ).reshape(4, P, F).transpose(1, 0, 2).reshape(P, 4 * F),
            "x8i": np.concatenate(
                [si8[d], si8[d ^ 1], si8[d ^ 2], si8[d ^ 4]]
            ).reshape(4, P, F).transpose(1, 0, 2).reshape(P, 4 * F),
            "xbr": srb[d].reshape(P, F),
            "xbi": sib[d].reshape(P, F),
            "wa7i": wa7i,
            "wia7": wia7,
            "wii": wii,
            "dlhs": dlhs,
            "drhs": drhs,
            "rh": rh_col,
        })

    nc = _build_program()
    trace = bool(int(os.environ.get("BASS_KERNEL_TRACE", "0")))
    kwargs = {}
    if trace:
        kwargs["tmpdir"] = os.environ.get("BASS_KERNEL_TRACE_DIR") or None
    res = run_bass_kernel_spmd(
        nc, in_maps, core_ids=list(range(N_CORES)), trace=trace, **kwargs)
    _cached["last_result"] = res

    out = np.empty((2, N_CORES * SHARD), np.float32)
    for d in range(N_CORES):
        out[0, d * SHARD:(d + 1) * SHARD] = res.results[d]["outr"].reshape(-1)
        out[1, d * SHARD:(d + 1) * SHARD] = res.results[d]["outi"].reshape(-1)
    return out


# revision 3
# speedup vs baseline: 1.6860x; 1.6860x over previous
"""Distributed Trainium2 kernel for the diagonal-Rydberg Hamiltonian apply.

Math (n = 22 qubits, dim = 2^22, psi complex as separate real/imag f32):
    out = (rabi/2) * sum_k flip_k(psi) + diag * psi
    diag(b) = sum_k (-detune) * bit_k(b) + sum_{i<j} triu(U,1)[i,j] bit_i(b) bit_j(b)

Distribution: state sharded over 8 cores along the 3 leading qubit axes.
Core d owns amplitudes with global index g = d (top 3 bits). Its output
needs its own shard plus the 3 Hamming-distance-1 partner shards.
All data each core needs is staged in its own DRAM; no collectives.

Per-core layout: local 19 bits -> [128 partitions (bits 12..18), 4096 free
(bits 0..11)]; free axis = 8 chunks of 512 columns (chunk bits 9..11).

Flip-sum strategy (fp8 terms, fp32 PSUM accumulation; 22 independent fp8
roundings stay ~1e-3 relative to the output scale):
  - PE does 6 fp8 DoubleRow matmuls + 1 bf16 matmul per chunk, each DR
    summing TWO k-tiles in one pass:
      DR1 [A7|I](own_c, own_c^1)   7 partition flips + chunk-bit-0 flip
      DR2 [I|I](own_c^2, own_c^4)  chunk-bit-1/2 flips
      DR3 [I|I](pb0_c, pb1_c)      partner shards d^1, d^2
      DR4 [I|I](pb2_c, j4_c)       partner d^4 + flip-copy j4
      DR5 [I|I](j5_c, j6_c)        flip-copies j5, j6
      DR6 [I|I](j7_c, j8_c)        flip-copies j7, j8
      MM7 [I](P_c bf16)            partial P = j0+j1+j2+j3
  - The j4..j7 flip tiles are produced by ACT strided copies (bf16 ->
    fp8 segments of the same SBUF tensor, so they ride DoubleRow); j8 is
    two contiguous half-swap SBUF->SBUF DMA copies; j0..j3 are two DVE
    pair-adds + one merge into a bf16 partial.
  - Producers run PIPE chunks ahead of the PE group so the PE stream
    never stalls (keeps the PE p-state ramped).
  - Diagonal: D built once by a K=9 float32r matmul from host bit
    tables; dx = D (.) x_bf16 on GPSIMD; finalize on DVE:
    out = psum * (rabi/2) + dx.
"""

import os
import sys

import numpy as np
import ml_dtypes

_REPO = "/opt/trn_rl_repo"
if _REPO not in sys.path:
    sys.path.insert(0, _REPO)

import concourse.mybir as mybir  # noqa: E402
from concourse import bacc  # noqa: E402
from concourse import bass  # noqa: E402
from concourse.tile import TileContext  # noqa: E402
from concourse.bass_utils import run_bass_kernel_spmd  # noqa: E402

N_Q = 22
N_GLOBAL = 3
N_CORES = 8
N_LOCAL = N_Q - N_GLOBAL          # 19
P_BITS = 7                        # partition bits (local bits 12..18)
F_BITS = N_LOCAL - P_BITS         # 12 free bits
P = 1 << P_BITS                   # 128
F = 1 << F_BITS                   # 4096
CHUNK = 512
N_CHUNKS = F // CHUNK             # 8
SHARD = P * F                     # 2^19
PIPE = 2                          # producer lookahead (chunks)

BF16 = ml_dtypes.bfloat16
FP8 = ml_dtypes.float8_e4m3

# fp8 SBUF tensor segments (units of F=4096 columns)
SEG_OWN, SEG_PB0, SEG_PB1, SEG_PB2 = 0, 1, 2, 3
SEG_J4, SEG_J5, SEG_J6, SEG_J7, SEG_J8 = 4, 5, 6, 7, 8
N_SEG = 9

_cached = {}


def _pair_ap(t, o1, o2, width=CHUNK):
    """Moving AP [128, 2, width] for a DoubleRow pair: k-tile0 at column o1,
    k-tile1 at column o2 of SBUF tile t. o2 > o1 required."""
    base = t[:, o1:o1 + width]
    d = o2 - o1
    assert d > 0
    return bass.AP(tensor=base.tensor, offset=base.offset,
                   ap=[list(base.ap[0]), [d, 2], [1, width]])


def _build_program():
    """Build the (input-independent) Bass program once per process."""
    if "nc" in _cached:
        return _cached["nc"]

    use_f32r = bool(int(os.environ.get("RYD_F32R", "1")))
    nc = bacc.Bacc("TRN2", num_devices=N_CORES)
    f32, bf16, fp8 = mybir.dt.float32, mybir.dt.bfloat16, mybir.dt.float8e4
    d_dt = mybir.dt.float32r if use_f32r else f32
    Alu = mybir.AluOpType

    x8r = nc.dram_tensor("x8r", [P, 4 * F], fp8, kind="ExternalInput")
    x8i = nc.dram_tensor("x8i", [P, 4 * F], fp8, kind="ExternalInput")
    xbr = nc.dram_tensor("xbr", [P, F], bf16, kind="ExternalInput")
    xbi = nc.dram_tensor("xbi", [P, F], bf16, kind="ExternalInput")
    wa7i = nc.dram_tensor("wa7i", [P, 2 * P], fp8, kind="ExternalInput")
    wia7 = nc.dram_tensor("wia7", [P, 2 * P], fp8, kind="ExternalInput")
    wii = nc.dram_tensor("wii", [P, 2 * P], fp8, kind="ExternalInput")
    wi16 = nc.dram_tensor("wi16", [P, P], bf16, kind="ExternalInput")
    dlhs = nc.dram_tensor("dlhs", [9, P], d_dt, kind="ExternalInput")
    drhs = nc.dram_tensor("drhs", [9, F], d_dt, kind="ExternalInput")
    rh = nc.dram_tensor("rh", [P, 1], f32, kind="ExternalInput")
    outr = nc.dram_tensor("outr", [P, F], f32, kind="ExternalOutput")
    outi = nc.dram_tensor("outi", [P, F], f32, kind="ExternalOutput")

    with TileContext(nc) as tc:
        with (
            tc.tile_pool(name="singles", bufs=1) as singles,
            tc.tile_pool(name="psum", bufs=6, space="PSUM") as psum_pool,
            tc.tile_pool(name="pp", bufs=3 * (PIPE + 2)) as pp_pool,
            tc.tile_pool(name="dx", bufs=PIPE + 2) as dx_pool,
            tc.tile_pool(name="osb", bufs=PIPE + 2) as osb_pool,
        ):
            # ---- aux loads ----
            t_wa7i = singles.tile([P, 2 * P], fp8, tag="wa7i")
            nc.sync.dma_start(out=t_wa7i[:], in_=wa7i[:])
            t_wia7 = singles.tile([P, 2 * P], fp8, tag="wia7")
            nc.sync.dma_start(out=t_wia7[:], in_=wia7[:])
            t_wii = singles.tile([P, 2 * P], fp8, tag="wii")
            nc.sync.dma_start(out=t_wii[:], in_=wii[:])
            t_wi16 = singles.tile([P, P], bf16, tag="wi16")
            nc.sync.dma_start(out=t_wi16[:], in_=wi16[:])
            t_dlhs = singles.tile([9, P], d_dt, tag="dlhs")
            nc.sync.dma_start(out=t_dlhs[:], in_=dlhs[:])
            t_drhs = singles.tile([9, F], d_dt, tag="drhs")
            nc.sync.dma_start(out=t_drhs[:], in_=drhs[:])
            t_rh = singles.tile([P, 1], f32, tag="rh")
            nc.sync.dma_start(out=t_rh[:], in_=rh[:])

            # ---- bulk loads, r-component first so its compute starts early
            t_x8, t_xb = {}, {}
            for name, d8, db16 in (("r", x8r, xbr), ("i", x8i, xbi)):
                t8 = singles.tile([P, N_SEG * F], fp8, tag=f"x8{name}")
                for s in range(4):
                    nc.sync.dma_start(out=t8[:, s * F:(s + 1) * F],
                                      in_=d8[:, s * F:(s + 1) * F])
                t_x8[name] = t8
                tb = singles.tile([P, F], bf16, tag=f"xb{name}")
                for h in range(2):
                    hs = slice(h * (F // 2), (h + 1) * (F // 2))
                    nc.sync.dma_start(out=tb[:, hs], in_=db16[:, hs])
                t_xb[name] = tb

            # ---- diagonal D = dlhs.T @ drhs (K=9), shared by r and i ----
            t_D = singles.tile([P, F], f32, tag="D")
            for c in range(N_CHUNKS):
                sl = slice(c * CHUNK, (c + 1) * CHUNK)
                pd = psum_pool.tile([P, CHUNK], f32, tag="psum")
                nc.tensor.matmul(pd[:], t_dlhs[:], t_drhs[:, sl],
                                 start=True, stop=True)
                nc.scalar.copy(t_D[:, sl], pd[:])

            # DoubleRow stationary views [K, 2, M]
            v_a7i = t_wa7i[:].rearrange("k (two m) -> k two m", two=2)
            v_ia7 = t_wia7[:].rearrange("k (two m) -> k two m", two=2)
            v_ii = t_wii[:].rearrange("k (two m) -> k two m", two=2)
            DR = mybir.MatmulPerfMode.DoubleRow

            # ---- main pipelined loop ----
            for name, out_dram in (("r", outr), ("i", outi)):
                x8 = t_x8[name]
                xb = t_xb[name]
                partials = {}

                def seg(s, c):
                    return s * F + c * CHUNK

                def flipv(j, c):
                    b = 1 << j
                    v = xb[:, c * CHUNK:(c + 1) * CHUNK].rearrange(
                        "p (g t b) -> p g t b", t=2, b=b)
                    return v[:, :, ::-1, :]

                def produce(c):
                    # DVE: j0..j3 partial (bf16)
                    p01 = pp_pool.tile([P, CHUNK], bf16, tag="p01")
                    nc.vector.tensor_add(out=p01[:], in0=flipv(0, c),
                                         in1=flipv(1, c))
                    p23 = pp_pool.tile([P, CHUNK], bf16, tag="p23")
                    nc.vector.tensor_add(out=p23[:], in0=flipv(2, c),
                                         in1=flipv(3, c))
                    pP = pp_pool.tile([P, CHUNK], bf16, tag="P")
                    nc.vector.tensor_add(out=pP[:], in0=p01[:], in1=p23[:])
                    partials[c] = pP
                    # ACT: flip-copies j4..j7 -> fp8 segments
                    for j, s in ((4, SEG_J4), (5, SEG_J5),
                                 (6, SEG_J6), (7, SEG_J7)):
                        nc.scalar.copy(
                            x8[:, seg(s, c):seg(s, c) + CHUNK], flipv(j, c))
                    # DMA: j8 = contiguous half-swap copies (fp8)
                    o = seg(SEG_OWN, c)
                    o8 = seg(SEG_J8, c)
                    H = CHUNK // 2
                    nc.sync.dma_start(out=x8[:, o8:o8 + H],
                                      in_=x8[:, o + H:o + CHUNK])
                    nc.sync.dma_start(out=x8[:, o8 + H:o8 + CHUNK],
                                      in_=x8[:, o:o + H])
                    # GPSIMD: diag product (f32 out)
                    dx = dx_pool.tile([P, CHUNK], f32, tag="dx")
                    sl = slice(c * CHUNK, (c + 1) * CHUNK)
                    nc.gpsimd.tensor_mul(out=dx[:], in0=t_D[:, sl],
                                         in1=xb[:, sl])
                    return dx

                def consume(c, dx):
                    acc = psum_pool.tile([P, CHUNK], f32, tag="psum")
                    c1 = c ^ 1
                    if c < c1:
                        nc.tensor.matmul(acc[:], v_a7i,
                                         _pair_ap(x8, c * CHUNK, c1 * CHUNK),
                                         start=True, stop=False,
                                         perf_mode=DR)
                    else:
                        nc.tensor.matmul(acc[:], v_ia7,
                                         _pair_ap(x8, c1 * CHUNK, c * CHUNK),
                                         start=True, stop=False,
                                         perf_mode=DR)
                    ca, cb = sorted((c ^ 2, c ^ 4))
                    nc.tensor.matmul(acc[:], v_ii,
                                     _pair_ap(x8, ca * CHUNK, cb * CHUNK),
                                     start=False, stop=False, perf_mode=DR)
                    nc.tensor.matmul(
                        acc[:], v_ii,
                        _pair_ap(x8, seg(SEG_PB0, c), seg(SEG_PB1, c)),
                        start=False, stop=False, perf_mode=DR)
                    nc.tensor.matmul(
                        acc[:], v_ii,
                        _pair_ap(x8, seg(SEG_PB2, c), seg(SEG_J4, c)),
                        start=False, stop=False, perf_mode=DR)
                    nc.tensor.matmul(
                        acc[:], v_ii,
                        _pair_ap(x8, seg(SEG_J5, c), seg(SEG_J6, c)),
                        start=False, stop=False, perf_mode=DR)
                    nc.tensor.matmul(
                        acc[:], v_ii,
                        _pair_ap(x8, seg(SEG_J7, c), seg(SEG_J8, c)),
                        start=False, stop=False, perf_mode=DR)
                    nc.tensor.matmul(acc[:], t_wi16[:], partials.pop(c)[:],
                                     start=False, stop=True)
                    # finalize: out = acc * (rabi/2) + dx
                    osb = osb_pool.tile([P, CHUNK], f32, tag="osb")
                    nc.vector.scalar_tensor_tensor(
                        out=osb[:], in0=acc[:], scalar=t_rh[:], in1=dx[:],
                        op0=Alu.mult, op1=Alu.add)
                    sl = slice(c * CHUNK, (c + 1) * CHUNK)
                    nc.sync.dma_start(out=out_dram[:, sl], in_=osb[:])

                dxs = {}
                for cc in range(N_CHUNKS + PIPE):
                    if cc < N_CHUNKS:
                        dxs[cc] = produce(cc)
                    if cc >= PIPE:
                        consume(cc - PIPE, dxs.pop(cc - PIPE))

    nc.finalize()
    _cached["nc"] = nc
    return nc


def _host_tables(U, detune, d):
    """Per-core diagonal tables for the K=9 on-device D matmul."""
    Ut = np.triu(U.astype(np.float64), 1)
    gval = {0: (d >> 2) & 1, 1: (d >> 1) & 1, 2: d & 1}  # qubit -> bit of d
    # linear coefficient for every local qubit (3..21)
    lin = np.zeros(N_Q)
    for q in range(3, N_Q):
        lin[q] = -detune + sum(gval[i] * Ut[i, q] for i in range(3))
    const_d = -detune * sum(gval.values())
    for i in range(3):
        for j in range(i + 1, 3):
            const_d += Ut[i, j] * gval[i] * gval[j]

    hi_q = [9 - m for m in range(P_BITS)]        # partition bit m -> qubit
    lo_q = [21 - r for r in range(F_BITS)]       # free bit r -> qubit

    pidx = np.arange(P)
    B7 = ((pidx[:, None] >> np.arange(P_BITS)[None, :]) & 1).astype(np.float64)
    fidx = np.arange(F)
    B12 = ((fidx[:, None] >> np.arange(F_BITS)[None, :]) & 1).astype(np.float64)

    def pair_coeff(qa, qb):
        return Ut[min(qa, qb), max(qa, qb)]

    M_hh = np.zeros((P_BITS, P_BITS))
    for m in range(P_BITS):
        for m2 in range(m + 1, P_BITS):
            M_hh[m, m2] = pair_coeff(hi_q[m], hi_q[m2])
    M_ll = np.zeros((F_BITS, F_BITS))
    for r in range(F_BITS):
        for r2 in range(r + 1, F_BITS):
            M_ll[r, r2] = pair_coeff(lo_q[r], lo_q[r2])
    cross = np.zeros((P_BITS, F_BITS))
    for m in range(P_BITS):
        for r in range(F_BITS):
            cross[m, r] = pair_coeff(hi_q[m], lo_q[r])

    T1 = const_d + B7 @ np.array([lin[q] for q in hi_q]) \
        + np.einsum("pm,mn,pn->p", B7, M_hh, B7)
    T2 = B12 @ np.array([lin[q] for q in lo_q]) \
        + np.einsum("fm,mn,fn->f", B12, M_ll, B12)

    dlhs = np.vstack([B7.T, np.ones((1, P)), T1[None, :]]).astype(np.float32)
    drhs = np.vstack([cross @ B12.T, T2[None, :],
                      np.ones((1, F))]).astype(np.float32)
    return dlhs, drhs


def kernel(state_real, state_imag, rabi, detune, U, n_qubits, **_unused):
    n = int(n_qubits)
    assert n == N_Q, f"kernel hardcoded for {N_Q} qubits, got {n}"
    sr = np.ascontiguousarray(np.asarray(state_real, np.float32)).reshape(
        N_CORES, SHARD)
    si = np.ascontiguousarray(np.asarray(state_imag, np.float32)).reshape(
        N_CORES, SHARD)
    rabi_f = float(np.asarray(rabi).reshape(-1)[0])
    det_f = float(np.asarray(detune).reshape(-1)[0])
    U_np = np.asarray(U, np.float32)

    sr8 = sr.astype(FP8)
    si8 = si.astype(FP8)
    srb = sr.astype(BF16)
    sib = si.astype(BF16)

    pidx = np.arange(P)
    A7 = (np.bitwise_count(pidx[:, None] ^ pidx[None, :]) == 1).astype(FP8)
    I128 = np.eye(P, dtype=FP8)
    wa7i = np.concatenate([A7, I128], axis=1)
    wia7 = np.concatenate([I128, A7], axis=1)
    wii = np.concatenate([I128, I128], axis=1)
    rh_col = np.full((P, 1), rabi_f * 0.5, np.float32)

    in_maps = []
    for d in range(N_CORES):
        dlhs, drhs = _host_tables(U_np, det_f, d)
        in_maps.append({
            "x8r": np.concatenate(
                [sr8[d], sr8[d ^ 1], sr8[d ^ 2], sr8[d ^ 4]]
            ).reshape(4, P, F).transpose(1, 0, 2).reshape(P, 4 * F),
            "x8i": np.concatenate(
                [si8[d], si8[d ^ 1], si8[d ^ 2], si8[d ^ 4]]
            ).reshape(4, P, F).transpose(1, 0, 2).reshape(P, 4 * F),
            "xbr": srb[d].reshape(P, F),
            "xbi": sib[d].reshape(P, F),
            "wa7i": wa7i,
            "wia7": wia7,
            "wii": wii,
            "wi16": np.eye(P, dtype=BF16),
            "dlhs": dlhs,
            "drhs": drhs,
            "rh": rh_col,
        })

    nc = _build_program()
    trace = bool(int(os.environ.get("BASS_KERNEL_TRACE", "0")))
    kwargs = {}
    if trace:
        kwargs["tmpdir"] = os.environ.get("BASS_KERNEL_TRACE_DIR") or None
    res = run_bass_kernel_spmd(
        nc, in_maps, core_ids=list(range(N_CORES)), trace=trace, **kwargs)
    _cached["last_result"] = res

    out = np.empty((2, N_CORES * SHARD), np.float32)
    for d in range(N_CORES):
        out[0, d * SHARD:(d + 1) * SHARD] = res.results[d]["outr"].reshape(-1)
        out[1, d * SHARD:(d + 1) * SHARD] = res.results[d]["outi"].reshape(-1)
    return out
